# revision 35
# baseline (speedup 1.0000x reference)
"""Trainium2 Bass kernel for DeterministicLSTMSensorBasedForwardDynamics.

Problem: B=4096, T=50, OBS=64, ACT=16, H=256, OUT=64, 5-layer MLP head.
  x = concat(traj, act)                     [B, T, 80]
  LSTM over T with silu cell activation (g = silu(zg), h = o * silu(c))
  MLP: 5 x (Dense(256) + silu), Dense(64)

Strategy (data parallel over 8 cores, 512 batch each):
  * Transposed layout: activations are [feature, batch]; weights are the
    PE-stationary operand, batch streams as the matmul moving dimension.
  * The 512 batch is split into TWO phase-shifted chunks of 256.  Each
    chunk's z^T needs only 4 PSUM banks (zg: 1 bank, zfio: 3 banks), so
    while chunk A is in its gate/DVE ladder, the PE runs chunk B's
    matmuls, and vice versa.  This removes the per-step PE stall of the
    single-chunk design (Wi of step t+1 had to wait for ALL gate reads
    of step t) and keeps the PE stream dense so its p-state clock ramps
    to 2.4 GHz (a sparse stream runs at the 1.2 GHz mid p-state).
  * Each 2KB bank holds TWO 256-wide m-tile accumulation regions: only
    the bank's first matmul uses start=True (the bank-wide pending-zero
    mark lets the second region's first write overwrite-where-pending)
    and only the bank's last matmul uses stop=True.
  * Gates: one tanh op over the f,i,o banks (tanh and silu share one
    ACT table set; sigmoid does not, hence sigmoid via tanh), then ONE
    4x-mode tensor_scalar sig = (tau+1)*0.5 turns all taus into true
    sigmoids, so every gate product is a cheap 2x-mode tensor_tensor
    (scalar_tensor_tensor only has a 1x uop - measured 683ns vs 417).
  * Per-chunk state is a combined bf16 [c | g] tile so u = sig_f*c and
    p = sig_i*g fuse into ONE FD1024 TT; c = up_f + up_i.  sig_o is
    produced after c (only h needs it).  h is written r0-first so the
    next step's k0 matmuls (contracting h rows 0:127) start early.
  * The second chunk's sc/h pair is deferred into the next step's
    emission slot so its DVE ladder is covered by the other chunk's
    ACT work; Wh k-matmuls run f,i,o banks first, g banks last, which
    closes tanh's input after 12 of 16 matmuls.
  * bh is folded into the Wi matmul via an all-ones input row (K=81).
  * Host prep builds x^T as bf16 [128, 2*T*256]: partitions = padded
    input feature, free = (chunk, t, b).  Everything runs in bf16 with
    fp32 PSUM accumulation (validated rel err 0.009 < 2e-2 tolerance).
  * ACT reads of PSUM must be bank-contiguous (strided cross-bank ACT
    access patterns crash the device); zfio is 3 whole adjacent banks.
  * DMA order: wi + a tiny 2-step x block per chunk first (step 0 gates
    on them), wh next, then the rest of x in 10-step blocks, MLP
    weights last.  Measured on-device exec: ~363us (baseline 552us).
"""

import sys

sys.path.insert(0, "/opt/trn_rl_repo")

import numpy as np
import ml_dtypes

import concourse.bacc as bacc
import concourse.tile as tile
from concourse import mybir
from concourse import bass_utils

N_CORES = 8
B, T, OBS, ACTD, H, OUT, NL = 4096, 50, 64, 16, 256, 64, 5
BC = B // N_CORES          # batch per core = 512
NCH = 2                    # pipelined chunks per core
CB = BC // NCH             # chunk batch = 256

BF16 = mybir.dt.bfloat16
F32 = mybir.dt.float32
F32R = mybir.dt.float32r
AF = mybir.ActivationFunctionType
ALU = mybir.AluOpType

# gate permutation: reference order (i, f, g, o) -> bank order (g, f, i, o)
_PERM = np.concatenate([np.arange(512, 768), np.arange(256, 512),
                        np.arange(0, 256), np.arange(768, 1024)])

_CACHE = {}


def _build(t_steps=T):
    """Build + compile the Bass module (cached)."""
    if t_steps in _CACHE:
        return _CACHE[t_steps]

    nc = bacc.Bacc("TRN2", target_bir_lowering=False, debug=False,
                   num_devices=N_CORES)

    xt_d = nc.dram_tensor("xt", [128, NCH * t_steps * CB], BF16,
                          kind="ExternalInput").ap()
    wh_d = nc.dram_tensor("wh", [128, 2048], BF16, kind="ExternalInput").ap()
    wi_d = nc.dram_tensor("wi", [128, 1024], BF16, kind="ExternalInput").ap()
    mlpw_d = nc.dram_tensor("mlpw", [128, NL * 2 * 256], BF16,
                            kind="ExternalInput").ap()
    mlpb_d = nc.dram_tensor("mlpb", [128, NL * 2], F32,
                            kind="ExternalInput").ap()
    wout_d = nc.dram_tensor("wout", [128, 128], BF16, kind="ExternalInput").ap()
    boutb_d = nc.dram_tensor("boutb", [128, 256], F32,
                             kind="ExternalInput").ap()
    pred_d = nc.dram_tensor("pred", [BC, OUT], F32, kind="ExternalOutput").ap()

    with tile.TileContext(nc) as tc:
        with (
            tc.tile_pool(name="singles", bufs=1) as singles,
            tc.tile_pool(name="hpool", bufs=3) as hpool,
            tc.tile_pool(name="work", bufs=4) as work,
            tc.tile_pool(name="psum", bufs=1, space="PSUM") as psum,
        ):
            # ---- weights / persistent state ----
            # DMA priority: wi + the first x block of each chunk gate step
            # 0, so they go first; wh is only needed from step 1, the MLP
            # weights only at the very end.
            # step-0-critical DMAs issue on FOUR different engine queues so
            # their ~0.65us DGE issue costs overlap instead of serializing
            wi = singles.tile([128, 1024], BF16, tag="wi")
            nc.sync.dma_start(wi[:], wi_d[:])

            xt = singles.tile([128, NCH * t_steps * CB], BF16, tag="xt")
            TBLK = 10 if t_steps % 10 == 0 else t_steps
            # tiny first block (2 steps) so step 0 starts ASAP
            T0B = min(2, TBLK)
            nc.scalar.dma_start(xt[:, 0:T0B * CB], xt_d[:, 0:T0B * CB])
            lo1 = t_steps * CB
            nc.gpsimd.dma_start(xt[:, lo1:lo1 + T0B * CB],
                                xt_d[:, lo1:lo1 + T0B * CB])

            wh = singles.tile([128, 2048], BF16, tag="wh")
            nc.sync.dma_start(wh[:], wh_d[:])

            for ch in range(NCH):
                lo = (ch * t_steps + T0B) * CB
                hi = (ch * t_steps + TBLK) * CB
                if hi > lo:
                    nc.sync.dma_start(xt[:, lo:hi], xt_d[:, lo:hi])
            for t0 in range(TBLK, t_steps, TBLK):
                for ch in range(NCH):
                    lo = (ch * t_steps + t0) * CB
                    hi = lo + TBLK * CB
                    nc.sync.dma_start(xt[:, lo:hi], xt_d[:, lo:hi])

            mlpw = singles.tile([128, NL * 2 * 256], BF16, tag="mlpw")
            nc.sync.dma_start(mlpw[:], mlpw_d[:])
            mlpb = singles.tile([128, NL * 2], F32, tag="mlpb")
            nc.sync.dma_start(mlpb[:], mlpb_d[:])
            wout = singles.tile([128, 128], BF16, tag="wout")
            nc.sync.dma_start(wout[:], wout_d[:])
            boutb = singles.tile([128, 256], F32, tag="boutb")
            nc.sync.dma_start(boutb[:], boutb_d[:])
            xt_r = xt[:].rearrange("p (ch t b) -> p ch t b", ch=NCH,
                                   t=t_steps)

            # per-chunk [c | g] combined state, bf16: c at [0:512), the
            # step's g at [512:1024) so the u/p products fuse into one
            # FD1024 2x-mode tensor_tensor (written at t=0, no memset)
            cg = [singles.tile([128, 1024], BF16, tag=f"cg{ch}",
                               name=f"cg{ch}") for ch in range(NCH)]
            # h_final for the MLP: free = (ktile, chunk, b)
            hlast = singles.tile([128, 1024], BF16, tag="hlast")

            # per-chunk rolling state handles
            hprev = [None, None]
            # deferred (sc, h) producers for the previous step, per chunk
            sig_sc = [None, None]   # (sig tile, t) pending sc+h emission

            def mm_phase(t, ch):
                """Wi (open groups) + Wh k0/k1 matmuls for (t, ch)."""
                zg = psum.tile([128, 512], F32, tag=f"zg{ch}",
                               name=f"zg{ch}")
                zfio = psum.tile([128, 1536], F32, tag=f"zfio{ch}",
                                 name=f"zfio{ch}")

                def dst(m):
                    if m < 2:
                        return zg[:, m * 256:(m + 1) * 256]
                    return zfio[:, (m - 2) * 256:(m - 1) * 256]

                # each 2KB bank holds TWO 256-wide m-tile regions: only the
                # bank's FIRST matmul starts (bank-wide pending-zero mark;
                # the second region's first write then overwrites-where-
                # pending), and only the bank's LAST matmul stops.
                rhs_x = xt_r[0:81, ch, t:t + 1, :]
                for m in range(8):
                    nc.tensor.matmul(dst(m), wi[0:81, m * 128:(m + 1) * 128],
                                     rhs_x, start=(m % 2 == 0),
                                     stop=(t == 0 and m % 2 == 1))
                if t > 0:
                    # fio banks (m2..m7) first so tanh's input closes after
                    # 12 Wh matmuls instead of 16; g banks trail (silu_g is
                    # off the recurrence-critical path)
                    h = hprev[ch]
                    for k in range(2):
                        for m in (2, 3, 4, 5, 6, 7):
                            nc.tensor.matmul(
                                dst(m),
                                wh[:, k * 1024 + m * 128:
                                   k * 1024 + (m + 1) * 128],
                                h[:, k * CB:(k + 1) * CB],
                                start=False,
                                stop=(k == 1 and m % 2 == 1))
                    for k in range(2):
                        for m in (0, 1):
                            nc.tensor.matmul(
                                dst(m),
                                wh[:, k * 1024 + m * 128:
                                   k * 1024 + (m + 1) * 128],
                                h[:, k * CB:(k + 1) * CB],
                                start=False,
                                stop=(k == 1 and m % 2 == 1))
                return zg, zfio

            def g_phase(t, ch, zg):
                nc.scalar.activation(cg[ch][:, 512:1024], zg[:], AF.Silu)

            def tau_phase(t, ch, zfio):
                tau = work.tile([128, 1536], BF16, tag=f"tau{ch}",
                                name=f"tau{ch}")
                nc.scalar.activation(tau[:], zfio[:], AF.Tanh, scale=0.5)
                return tau

            def dve_phase(t, ch, tau):
                """sig = (tau+1)*0.5 (4x-mode tensor_scalar), then
                up = sig_fi * [c|g] (one FD1024 2x TT), c = up_f + up_i.
                sig_o is produced after c (it is only needed for h, so it
                stays off the recurrence-critical path)."""
                cg_t = cg[ch]
                sig = work.tile([128, 1536], BF16, tag=f"sig{ch}",
                                name=f"sig{ch}")
                nc.vector.tensor_scalar(sig[:, 0:1024], tau[:, 0:1024],
                                        1.0, 0.5, ALU.add, ALU.mult)
                if t > 0:
                    up = work.tile([128, 1024], BF16, tag=f"up{ch}",
                                   name=f"up{ch}")
                    nc.vector.tensor_mul(up[:], sig[:, 0:1024], cg_t[:])
                    nc.vector.tensor_add(cg_t[:, 0:512], up[:, 0:512],
                                         up[:, 512:1024])
                else:
                    # c = sig_i * g  (forget path dead at t=0)
                    nc.vector.tensor_mul(cg_t[:, 0:512], sig[:, 512:1024],
                                         cg_t[:, 512:1024])
                nc.vector.tensor_scalar(sig[:, 1024:1536],
                                        tau[:, 1024:1536],
                                        1.0, 0.5, ALU.add, ALU.mult)
                return sig

            def sc_h_phase(ch, t):
                """ACT: sc = silu(c); DVE: h = sig_o * sc.
                Deferred for chunk B into the next step's emission slot."""
                sig = sig_sc[ch][0]
                sc = work.tile([128, 512], BF16, tag=f"sc{ch}",
                               name=f"sc{ch}")
                nc.scalar.activation(sc[:], cg[ch][:, 0:512], AF.Silu)
                last = t == t_steps - 1
                if not last:
                    # r-split: h rows 0:128 first, so the next step's k0
                    # matmuls (which contract only those rows) start early
                    h_new = hpool.tile([128, 512], BF16, tag=f"h{ch}",
                                       name=f"h{ch}")
                    nc.vector.tensor_mul(h_new[:, 0:256],
                                         sig[:, 1024:1280], sc[:, 0:256])
                    nc.vector.tensor_mul(h_new[:, 256:512],
                                         sig[:, 1280:1536], sc[:, 256:512])
                    hprev[ch] = h_new
                else:
                    # final h -> hlast (fp32r), free = (ktile, chunk, b)
                    for r in range(2):
                        nc.vector.tensor_mul(
                            hlast[:, r * 512 + ch * 256:
                                  r * 512 + (ch + 1) * 256],
                            sig[:, 1024 + r * 256:1024 + (r + 1) * 256],
                            sc[:, r * 256:(r + 1) * 256])
                    hprev[ch] = None
                sig_sc[ch] = None

            # ---- LSTM over time, 2 phase-shifted chunks ----
            # The second chunk's sc/h is deferred into the next step's
            # emission slot so its DVE c-ladder is covered by the other
            # chunk's ACT work; roles alternate per step so the deferral
            # stall is not always paid by the same chunk.
            for t in range(t_steps):
                zgA, zfioA = mm_phase(t, 0)
                g_phase(t, 0, zgA)
                if sig_sc[1] is not None:
                    sc_h_phase(1, t - 1)
                tauA = tau_phase(t, 0, zfioA)
                zgB, zfioB = mm_phase(t, 1)
                # B's tanh BEFORE B's silu_g: the scheduler breaks ready
                # ties by emission order, and tanh-first lets B's DVE
                # ladder start ~0.7us earlier while g_B fills the ACT arc
                # before sc_B (B's ladder is the one exposed stall).
                tauB = tau_phase(t, 1, zfioB)
                g_phase(t, 1, zgB)
                sigA = dve_phase(t, 0, tauA)
                sig_sc[0] = (sigA, t)
                sc_h_phase(0, t)
                sigB = dve_phase(t, 1, tauB)
                sig_sc[1] = (sigB, t)
            # flush chunk B's final tail
            sc_h_phase(1, t_steps - 1)

            # ---- MLP head (fp32r, full 512 batch) ----
            cur = hlast
            for layer in range(NL):
                # separate psum tiles per m-tile so silu(m0) overlaps the
                # m1 matmuls instead of waiting on the whole layer
                mps = [psum.tile([128, 512], F32, tag=f"zg{m}",
                                 name=f"mlp_ps{layer}_{m}")
                       for m in range(2)]
                nxt = work.tile([128, 1024], BF16, tag="mlp_out")
                for m in range(2):
                    for k in range(2):
                        nc.tensor.matmul(
                            mps[m][:],
                            mlpw[:, (layer * 2 + k) * 256 + m * 128:
                                 (layer * 2 + k) * 256 + (m + 1) * 128
                                 ],
                            cur[:, k * 512:(k + 1) * 512],
                            start=(k == 0), stop=(k == 1))
                    nc.scalar.activation(
                        nxt[:, m * 512:(m + 1) * 512],
                        mps[m][:], AF.Silu,
                        bias=mlpb[:, layer * 2 + m:layer * 2 + m + 1])
                cur = nxt

            # output layer back to [batch, OUT] layout:
            # lhsT = activations (stationary), rhs = Wout (moving)
            pp = psum.tile([128, 256], F32, tag="zg0", name="pred_ps")
            for m in range(4):
                for k in range(2):
                    nc.tensor.matmul(
                        pp[:, m * 64:(m + 1) * 64],
                        cur[:, k * 512 + m * 128:k * 512 + (m + 1) * 128
                            ],
                        wout[:, k * 64:(k + 1) * 64],
                        start=(m == 0 and k == 0), stop=(m == 3 and k == 1))
            preds = singles.tile([128, 256], F32, tag="preds")
            nc.vector.tensor_add(preds[:], pp[:], boutb[:])
            nc.sync.dma_start(
                pred_d.rearrange("(m p) f -> p m f", p=128),
                preds[:].rearrange("p (m f) -> p m f", f=OUT))

    nc.compile()
    _CACHE[t_steps] = nc
    return nc


def _prep_inputs(trajectory, actions, Wi, Wh, bh, mlp_W, mlp_b, Wout, bout,
                 t_steps=T):
    """Host-side layout prep. Returns per-core input maps."""
    f32 = np.float32
    trajectory = np.asarray(trajectory, f32)
    actions = np.asarray(actions, f32)
    Wi = np.asarray(Wi, f32)
    Wh = np.asarray(Wh, f32)
    bh = np.asarray(bh, f32)
    mlp_W = np.asarray(mlp_W, f32)
    mlp_b = np.asarray(mlp_b, f32)
    Wout = np.asarray(Wout, f32)
    bout = np.asarray(bout, f32)

    # gate permutation (bank order g, f, i, o)
    Wh_p = Wh[:, _PERM].astype(ml_dtypes.bfloat16)
    Wi_p = Wi[:, _PERM]
    bh_p = bh[_PERM]

    wh_l = Wh_p.reshape(2, 128, 1024).transpose(1, 0, 2).reshape(128, 2048)
    wi_l = np.zeros((128, 1024), ml_dtypes.bfloat16)
    wi_l[0:OBS] = Wi_p[0:OBS].astype(ml_dtypes.bfloat16)
    wi_l[OBS:OBS + ACTD] = Wi_p[OBS:OBS + ACTD].astype(ml_dtypes.bfloat16)
    wi_l[80] = bh_p.astype(ml_dtypes.bfloat16)

    mlpw_l = mlp_W.reshape(NL, 2, 128, 256).transpose(2, 0, 1, 3).reshape(
        128, NL * 2 * 256).astype(ml_dtypes.bfloat16)
    mlpb_l = mlp_b.reshape(NL, 2, 128).transpose(2, 0, 1).reshape(128, NL * 2)
    wout_l = Wout.reshape(2, 128, 64).transpose(1, 0, 2).reshape(
        128, 128).astype(ml_dtypes.bfloat16)
    boutb_l = np.tile(bout, (128, 4))

    in_maps = []
    for c in range(N_CORES):
        tr = trajectory[c * BC:(c + 1) * BC, :t_steps]    # [512, t, 64]
        ac = actions[c * BC:(c + 1) * BC, :t_steps]       # [512, t, 16]
        xt = np.zeros((128, NCH, t_steps, CB), ml_dtypes.bfloat16)
        xt[0:OBS] = tr.reshape(NCH, CB, t_steps, OBS).transpose(
            3, 0, 2, 1).astype(ml_dtypes.bfloat16)
        xt[OBS:OBS + ACTD] = ac.reshape(NCH, CB, t_steps, ACTD).transpose(
            3, 0, 2, 1).astype(ml_dtypes.bfloat16)
        xt[80] = 1.0
        in_maps.append({
            "xt": xt.reshape(128, NCH * t_steps * CB),
            "wh": wh_l, "wi": wi_l, "mlpw": mlpw_l,
            "mlpb": mlpb_l.astype(f32), "wout": wout_l,
            "boutb": boutb_l.astype(f32),
        })
    return in_maps


_RUNNER = {}


def _get_runner(t_steps=T):
    """Build the bass module once and wrap it in a cached, reusable
    shard-mapped PJRT executable (one NEFF compile per process)."""
    if t_steps in _RUNNER:
        return _RUNNER[t_steps]

    import jax
    from jax.sharding import Mesh, PartitionSpec
    from jax.experimental.shard_map import shard_map
    from concourse import bass2jax, mybir as _mb

    nc = _build(t_steps)
    bass2jax.install_neuronx_cc_hook()

    part_name = (nc.partition_id_tensor.name if nc.partition_id_tensor
                 else None)
    in_names, out_names, out_avals = [], [], []
    for alloc in nc.m.functions[0].allocations:
        if not isinstance(alloc, _mb.MemoryLocationSet):
            continue
        name = alloc.memorylocations[0].name
        if alloc.kind == "ExternalInput":
            if name != part_name:
                in_names.append(name)
        elif alloc.kind == "ExternalOutput":
            out_names.append(name)
            out_avals.append(jax.core.ShapedArray(
                tuple(alloc.tensor_shape), _mb.dt.np(alloc.dtype)))
    n_params = len(in_names)
    n_outs = len(out_avals)
    all_names = in_names + out_names
    if part_name is not None:
        all_names = all_names + [part_name]

    def _body(*args):
        operands = list(args)
        if part_name is not None:
            operands.append(bass2jax.partition_id_tensor())
        outs = bass2jax._bass_exec_p.bind(
            *operands,
            out_avals=tuple(out_avals),
            in_names=tuple(all_names),
            out_names=tuple(out_names),
            lowering_input_output_aliases=(),
            sim_require_finite=True,
            sim_require_nnan=True,
            nc=nc,
        )
        return tuple(outs)

    devices = jax.devices()[:N_CORES]
    mesh = Mesh(np.asarray(devices), ("core",))
    donate = tuple(range(n_params, n_params + n_outs))
    sharded = jax.jit(
        shard_map(_body, mesh=mesh,
                  in_specs=(PartitionSpec("core"),) * (n_params + n_outs),
                  out_specs=(PartitionSpec("core"),) * n_outs,
                  check_rep=False),
        donate_argnums=donate, keep_unused=True)

    sharded_nodon = jax.jit(
        shard_map(_body, mesh=mesh,
                  in_specs=(PartitionSpec("core"),) * (n_params + n_outs),
                  out_specs=(PartitionSpec("core"),) * n_outs,
                  check_rep=False),
        keep_unused=True)

    out_shapes = [(a.shape, a.dtype) for a in out_avals]

    def run(in_maps):
        concat_in = [
            np.concatenate([np.asarray(in_maps[c][nm]) for c in
                            range(N_CORES)], axis=0)
            for nm in in_names
        ]
        zeros = [np.zeros((N_CORES * s[0],) + tuple(s[1:]), dt)
                 for s, dt in out_shapes]
        outs = sharded(*concat_in, *zeros)
        return {nm: np.asarray(outs[i]) for i, nm in enumerate(out_names)}

    run.in_names = in_names
    run.mesh = mesh
    run.nodon = sharded_nodon
    run.out_shapes = out_shapes
    _RUNNER[t_steps] = run
    return run


def _stage_inputs(in_maps, t_steps=T):
    """device_put concatenated inputs + zero outs once, for repeat timing."""
    import jax
    from jax.sharding import NamedSharding, PartitionSpec
    run = _get_runner(t_steps)
    sh = NamedSharding(run.mesh, PartitionSpec("core"))
    concat_in = [
        np.concatenate([np.asarray(in_maps[c][nm]) for c in range(N_CORES)],
                       axis=0)
        for nm in run.in_names
    ]
    zeros = [np.zeros((N_CORES * s[0],) + tuple(s[1:]), dt)
             for s, dt in run.out_shapes]
    return [jax.device_put(a, sh) for a in concat_in + zeros], run


def _run_staged(staged):
    arrs, run = staged
    return run.nodon(*arrs)


def kernel(trajectory, actions, Wi, Wh, bh, mlp_W, mlp_b, Wout, bout):
    run = _get_runner(T)
    in_maps = _prep_inputs(trajectory, actions, Wi, Wh, bh, mlp_W, mlp_b,
                           Wout, bout, T)
    pred = run(in_maps)["pred"]          # [8*512, 64] already batch-ordered
    return pred.astype(np.float32)


# revision 36
# speedup vs baseline: 1.0005x; 1.0005x over previous
"""Trainium2 Bass kernel for DeterministicLSTMSensorBasedForwardDynamics.

Problem: B=4096, T=50, OBS=64, ACT=16, H=256, OUT=64, 5-layer MLP head.
  x = concat(traj, act)                     [B, T, 80]
  LSTM over T with silu cell activation (g = silu(zg), h = o * silu(c))
  MLP: 5 x (Dense(256) + silu), Dense(64)

Strategy (data parallel over 8 cores, 512 batch each):
  * Transposed layout: activations are [feature, batch]; weights are the
    PE-stationary operand, batch streams as the matmul moving dimension.
  * The 512 batch is split into TWO phase-shifted chunks of 256.  Each
    chunk's z^T needs only 4 PSUM banks (zg: 1 bank, zfio: 3 banks), so
    while chunk A is in its gate/DVE ladder, the PE runs chunk B's
    matmuls, and vice versa.  This removes the per-step PE stall of the
    single-chunk design (Wi of step t+1 had to wait for ALL gate reads
    of step t) and keeps the PE stream dense so its p-state clock ramps
    to 2.4 GHz (a sparse stream runs at the 1.2 GHz mid p-state).
  * Each 2KB bank holds TWO 256-wide m-tile accumulation regions: only
    the bank's first matmul uses start=True (the bank-wide pending-zero
    mark lets the second region's first write overwrite-where-pending)
    and only the bank's last matmul uses stop=True.
  * Gates: one tanh op over the f,i,o banks (tanh and silu share one
    ACT table set; sigmoid does not, hence sigmoid via tanh), then ONE
    4x-mode tensor_scalar sig = (tau+1)*0.5 turns all taus into true
    sigmoids, so every gate product is a cheap 2x-mode tensor_tensor
    (scalar_tensor_tensor only has a 1x uop - measured 683ns vs 417).
  * Per-chunk state is a combined bf16 [c | g] tile so u = sig_f*c and
    p = sig_i*g fuse into ONE FD1024 TT; c = up_f + up_i.  sig_o is
    produced after c (only h needs it).  h is written r0-first so the
    next step's k0 matmuls (contracting h rows 0:127) start early.
  * The second chunk's sc/h pair is deferred into the next step's
    emission slot so its DVE ladder is covered by the other chunk's
    ACT work; Wh k-matmuls run f,i,o banks first, g banks last, which
    closes tanh's input after 12 of 16 matmuls.
  * bh is folded into the Wi matmul via an all-ones input row (K=81).
  * Host prep builds x^T as bf16 [128, 2*T*256]: partitions = padded
    input feature, free = (chunk, t, b).  Everything runs in bf16 with
    fp32 PSUM accumulation (validated rel err 0.009 < 2e-2 tolerance).
  * ACT reads of PSUM must be bank-contiguous (strided cross-bank ACT
    access patterns crash the device); zfio is 3 whole adjacent banks.
  * DMA order: wi + a tiny 2-step x block per chunk first (step 0 gates
    on them), wh next, then the rest of x in 10-step blocks, MLP
    weights last.  Measured on-device exec: ~363us (baseline 552us).
"""

import sys

sys.path.insert(0, "/opt/trn_rl_repo")

import numpy as np
import ml_dtypes

import concourse.bacc as bacc
import concourse.tile as tile
from concourse import mybir
from concourse import bass_utils

N_CORES = 8
B, T, OBS, ACTD, H, OUT, NL = 4096, 50, 64, 16, 256, 64, 5
BC = B // N_CORES          # batch per core = 512
NCH = 2                    # pipelined chunks per core
CB = BC // NCH             # chunk batch = 256

BF16 = mybir.dt.bfloat16
F32 = mybir.dt.float32
F32R = mybir.dt.float32r
AF = mybir.ActivationFunctionType
ALU = mybir.AluOpType

# gate permutation: reference order (i, f, g, o) -> bank order (g, f, i, o)
_PERM = np.concatenate([np.arange(512, 768), np.arange(256, 512),
                        np.arange(0, 256), np.arange(768, 1024)])

_CACHE = {}


def _build(t_steps=T):
    """Build + compile the Bass module (cached)."""
    if t_steps in _CACHE:
        return _CACHE[t_steps]

    nc = bacc.Bacc("TRN2", target_bir_lowering=False, debug=False,
                   num_devices=N_CORES)

    xt_d = nc.dram_tensor("xt", [128, NCH * t_steps * CB], BF16,
                          kind="ExternalInput").ap()
    wh_d = nc.dram_tensor("wh", [128, 2048], BF16, kind="ExternalInput").ap()
    wi_d = nc.dram_tensor("wi", [128, 1024], BF16, kind="ExternalInput").ap()
    mlpw_d = nc.dram_tensor("mlpw", [128, NL * 2 * 256], BF16,
                            kind="ExternalInput").ap()
    mlpb_d = nc.dram_tensor("mlpb", [128, NL * 2], F32,
                            kind="ExternalInput").ap()
    wout_d = nc.dram_tensor("wout", [128, 128], BF16, kind="ExternalInput").ap()
    boutb_d = nc.dram_tensor("boutb", [128, 256], F32,
                             kind="ExternalInput").ap()
    pred_d = nc.dram_tensor("pred", [BC, OUT], F32, kind="ExternalOutput").ap()

    with tile.TileContext(nc) as tc:
        with (
            tc.tile_pool(name="singles", bufs=1) as singles,
            tc.tile_pool(name="hpool", bufs=3) as hpool,
            tc.tile_pool(name="work", bufs=3) as work,
            tc.tile_pool(name="psum", bufs=1, space="PSUM") as psum,
        ):
            # ---- weights / persistent state ----
            # DMA priority: wi + the first x block of each chunk gate step
            # 0, so they go first; wh is only needed from step 1, the MLP
            # weights only at the very end.
            # step-0-critical DMAs issue on FOUR different engine queues so
            # their ~0.65us DGE issue costs overlap instead of serializing
            wi = singles.tile([128, 1024], BF16, tag="wi")
            nc.sync.dma_start(wi[:], wi_d[:])

            xt = singles.tile([128, NCH * t_steps * CB], BF16, tag="xt")
            TBLK = 10 if t_steps % 10 == 0 else t_steps
            # tiny first block (2 steps) so step 0 starts ASAP
            T0B = min(2, TBLK)
            nc.scalar.dma_start(xt[:, 0:T0B * CB], xt_d[:, 0:T0B * CB])
            lo1 = t_steps * CB
            nc.gpsimd.dma_start(xt[:, lo1:lo1 + T0B * CB],
                                xt_d[:, lo1:lo1 + T0B * CB])

            wh = singles.tile([128, 2048], BF16, tag="wh")
            nc.sync.dma_start(wh[:], wh_d[:])

            for ch in range(NCH):
                lo = (ch * t_steps + T0B) * CB
                hi = (ch * t_steps + TBLK) * CB
                if hi > lo:
                    nc.sync.dma_start(xt[:, lo:hi], xt_d[:, lo:hi])
            for t0 in range(TBLK, t_steps, TBLK):
                for ch in range(NCH):
                    lo = (ch * t_steps + t0) * CB
                    hi = lo + TBLK * CB
                    nc.sync.dma_start(xt[:, lo:hi], xt_d[:, lo:hi])

            mlpw = singles.tile([128, NL * 2 * 256], BF16, tag="mlpw")
            nc.sync.dma_start(mlpw[:], mlpw_d[:])
            mlpb = singles.tile([128, NL * 2], F32, tag="mlpb")
            nc.sync.dma_start(mlpb[:], mlpb_d[:])
            wout = singles.tile([128, 128], BF16, tag="wout")
            nc.sync.dma_start(wout[:], wout_d[:])
            boutb = singles.tile([128, 256], F32, tag="boutb")
            nc.sync.dma_start(boutb[:], boutb_d[:])
            xt_r = xt[:].rearrange("p (ch t b) -> p ch t b", ch=NCH,
                                   t=t_steps)

            # per-chunk [c | g] combined state, bf16: c at [0:512), the
            # step's g at [512:1024) so the u/p products fuse into one
            # FD1024 2x-mode tensor_tensor (written at t=0, no memset)
            cg = [singles.tile([128, 1024], BF16, tag=f"cg{ch}",
                               name=f"cg{ch}") for ch in range(NCH)]
            # h_final for the MLP: free = (ktile, chunk, b)
            hlast = singles.tile([128, 1024], BF16, tag="hlast")

            # per-chunk rolling state handles
            hprev = [None, None]
            # deferred (sc, h) producers for the previous step, per chunk
            sig_sc = [None, None]   # (sig tile, t) pending sc+h emission

            def mm_phase(t, ch):
                """Wi (open groups) + Wh k0/k1 matmuls for (t, ch)."""
                zg = psum.tile([128, 512], F32, tag=f"zg{ch}",
                               name=f"zg{ch}")
                zfio = psum.tile([128, 1536], F32, tag=f"zfio{ch}",
                                 name=f"zfio{ch}")

                def dst(m):
                    if m < 2:
                        return zg[:, m * 256:(m + 1) * 256]
                    return zfio[:, (m - 2) * 256:(m - 1) * 256]

                # each 2KB bank holds TWO 256-wide m-tile regions: only the
                # bank's FIRST matmul starts (bank-wide pending-zero mark;
                # the second region's first write then overwrites-where-
                # pending), and only the bank's LAST matmul stops.
                rhs_x = xt_r[0:81, ch, t:t + 1, :]
                for m in range(8):
                    nc.tensor.matmul(dst(m), wi[0:81, m * 128:(m + 1) * 128],
                                     rhs_x, start=(m % 2 == 0),
                                     stop=(t == 0 and m % 2 == 1))
                if t > 0:
                    # fio banks (m2..m7) first so tanh's input closes after
                    # 12 Wh matmuls instead of 16; g banks trail (silu_g is
                    # off the recurrence-critical path)
                    h = hprev[ch]
                    for k in range(2):
                        for m in (2, 3, 4, 5, 6, 7):
                            nc.tensor.matmul(
                                dst(m),
                                wh[:, k * 1024 + m * 128:
                                   k * 1024 + (m + 1) * 128],
                                h[:, k * CB:(k + 1) * CB],
                                start=False,
                                stop=(k == 1 and m % 2 == 1))
                    for k in range(2):
                        for m in (0, 1):
                            nc.tensor.matmul(
                                dst(m),
                                wh[:, k * 1024 + m * 128:
                                   k * 1024 + (m + 1) * 128],
                                h[:, k * CB:(k + 1) * CB],
                                start=False,
                                stop=(k == 1 and m % 2 == 1))
                return zg, zfio

            def g_phase(t, ch, zg):
                nc.scalar.activation(cg[ch][:, 512:1024], zg[:], AF.Silu)

            def tau_phase(t, ch, zfio):
                tau = work.tile([128, 1536], BF16, tag=f"tau{ch}",
                                name=f"tau{ch}")
                nc.scalar.activation(tau[:], zfio[:], AF.Tanh, scale=0.5)
                return tau

            def dve_phase(t, ch, tau):
                """sig = (tau+1)*0.5 (4x-mode tensor_scalar), then
                up = sig_fi * [c|g] (one FD1024 2x TT), c = up_f + up_i.
                sig_o is produced after c (it is only needed for h, so it
                stays off the recurrence-critical path)."""
                cg_t = cg[ch]
                sig = work.tile([128, 1536], BF16, tag=f"sig{ch}",
                                name=f"sig{ch}")
                nc.vector.tensor_scalar(sig[:, 0:1024], tau[:, 0:1024],
                                        1.0, 0.5, ALU.add, ALU.mult)
                if t > 0:
                    up = work.tile([128, 1024], BF16, tag=f"up{ch}",
                                   name=f"up{ch}")
                    nc.vector.tensor_mul(up[:], sig[:, 0:1024], cg_t[:])
                    nc.vector.tensor_add(cg_t[:, 0:512], up[:, 0:512],
                                         up[:, 512:1024])
                else:
                    # c = sig_i * g  (forget path dead at t=0)
                    nc.vector.tensor_mul(cg_t[:, 0:512], sig[:, 512:1024],
                                         cg_t[:, 512:1024])
                nc.vector.tensor_scalar(sig[:, 1024:1536],
                                        tau[:, 1024:1536],
                                        1.0, 0.5, ALU.add, ALU.mult)
                return sig

            def sc_h_phase(ch, t):
                """ACT: sc = silu(c); DVE: h = sig_o * sc.
                Deferred for chunk B into the next step's emission slot."""
                sig = sig_sc[ch][0]
                sc = work.tile([128, 512], BF16, tag=f"sc{ch}",
                               name=f"sc{ch}")
                nc.scalar.activation(sc[:], cg[ch][:, 0:512], AF.Silu)
                last = t == t_steps - 1
                if not last:
                    # r-split: h rows 0:128 first, so the next step's k0
                    # matmuls (which contract only those rows) start early
                    h_new = hpool.tile([128, 512], BF16, tag=f"h{ch}",
                                       name=f"h{ch}")
                    nc.vector.tensor_mul(h_new[:, 0:256],
                                         sig[:, 1024:1280], sc[:, 0:256])
                    nc.vector.tensor_mul(h_new[:, 256:512],
                                         sig[:, 1280:1536], sc[:, 256:512])
                    hprev[ch] = h_new
                else:
                    # final h -> hlast (fp32r), free = (ktile, chunk, b)
                    for r in range(2):
                        nc.vector.tensor_mul(
                            hlast[:, r * 512 + ch * 256:
                                  r * 512 + (ch + 1) * 256],
                            sig[:, 1024 + r * 256:1024 + (r + 1) * 256],
                            sc[:, r * 256:(r + 1) * 256])
                    hprev[ch] = None
                sig_sc[ch] = None

            # ---- LSTM over time, 2 phase-shifted chunks ----
            # The second chunk's sc/h is deferred into the next step's
            # emission slot so its DVE c-ladder is covered by the other
            # chunk's ACT work; roles alternate per step so the deferral
            # stall is not always paid by the same chunk.
            for t in range(t_steps):
                zgA, zfioA = mm_phase(t, 0)
                g_phase(t, 0, zgA)
                if sig_sc[1] is not None:
                    sc_h_phase(1, t - 1)
                tauA = tau_phase(t, 0, zfioA)
                zgB, zfioB = mm_phase(t, 1)
                # B's tanh BEFORE B's silu_g: the scheduler breaks ready
                # ties by emission order, and tanh-first lets B's DVE
                # ladder start ~0.7us earlier while g_B fills the ACT arc
                # before sc_B (B's ladder is the one exposed stall).
                tauB = tau_phase(t, 1, zfioB)
                g_phase(t, 1, zgB)
                sigA = dve_phase(t, 0, tauA)
                sig_sc[0] = (sigA, t)
                sc_h_phase(0, t)
                sigB = dve_phase(t, 1, tauB)
                sig_sc[1] = (sigB, t)
            # flush chunk B's final tail
            sc_h_phase(1, t_steps - 1)

            # ---- MLP head (fp32r, full 512 batch) ----
            cur = hlast
            for layer in range(NL):
                # separate psum tiles per m-tile so silu(m0) overlaps the
                # m1 matmuls instead of waiting on the whole layer
                mps = [psum.tile([128, 512], F32, tag=f"zg{m}",
                                 name=f"mlp_ps{layer}_{m}")
                       for m in range(2)]
                nxt = work.tile([128, 1024], BF16, tag="mlp_out")
                for m in range(2):
                    for k in range(2):
                        nc.tensor.matmul(
                            mps[m][:],
                            mlpw[:, (layer * 2 + k) * 256 + m * 128:
                                 (layer * 2 + k) * 256 + (m + 1) * 128
                                 ],
                            cur[:, k * 512:(k + 1) * 512],
                            start=(k == 0), stop=(k == 1))
                    nc.scalar.activation(
                        nxt[:, m * 512:(m + 1) * 512],
                        mps[m][:], AF.Silu,
                        bias=mlpb[:, layer * 2 + m:layer * 2 + m + 1])
                cur = nxt

            # output layer back to [batch, OUT] layout:
            # lhsT = activations (stationary), rhs = Wout (moving)
            pp = psum.tile([128, 256], F32, tag="zg0", name="pred_ps")
            for m in range(4):
                for k in range(2):
                    nc.tensor.matmul(
                        pp[:, m * 64:(m + 1) * 64],
                        cur[:, k * 512 + m * 128:k * 512 + (m + 1) * 128
                            ],
                        wout[:, k * 64:(k + 1) * 64],
                        start=(m == 0 and k == 0), stop=(m == 3 and k == 1))
            preds = singles.tile([128, 256], F32, tag="preds")
            nc.vector.tensor_add(preds[:], pp[:], boutb[:])
            nc.sync.dma_start(
                pred_d.rearrange("(m p) f -> p m f", p=128),
                preds[:].rearrange("p (m f) -> p m f", f=OUT))

    nc.compile()
    _CACHE[t_steps] = nc
    return nc


def _prep_inputs(trajectory, actions, Wi, Wh, bh, mlp_W, mlp_b, Wout, bout,
                 t_steps=T):
    """Host-side layout prep. Returns per-core input maps."""
    f32 = np.float32
    trajectory = np.asarray(trajectory, f32)
    actions = np.asarray(actions, f32)
    Wi = np.asarray(Wi, f32)
    Wh = np.asarray(Wh, f32)
    bh = np.asarray(bh, f32)
    mlp_W = np.asarray(mlp_W, f32)
    mlp_b = np.asarray(mlp_b, f32)
    Wout = np.asarray(Wout, f32)
    bout = np.asarray(bout, f32)

    # gate permutation (bank order g, f, i, o)
    Wh_p = Wh[:, _PERM].astype(ml_dtypes.bfloat16)
    Wi_p = Wi[:, _PERM]
    bh_p = bh[_PERM]

    wh_l = Wh_p.reshape(2, 128, 1024).transpose(1, 0, 2).reshape(128, 2048)
    wi_l = np.zeros((128, 1024), ml_dtypes.bfloat16)
    wi_l[0:OBS] = Wi_p[0:OBS].astype(ml_dtypes.bfloat16)
    wi_l[OBS:OBS + ACTD] = Wi_p[OBS:OBS + ACTD].astype(ml_dtypes.bfloat16)
    wi_l[80] = bh_p.astype(ml_dtypes.bfloat16)

    mlpw_l = mlp_W.reshape(NL, 2, 128, 256).transpose(2, 0, 1, 3).reshape(
        128, NL * 2 * 256).astype(ml_dtypes.bfloat16)
    mlpb_l = mlp_b.reshape(NL, 2, 128).transpose(2, 0, 1).reshape(128, NL * 2)
    wout_l = Wout.reshape(2, 128, 64).transpose(1, 0, 2).reshape(
        128, 128).astype(ml_dtypes.bfloat16)
    boutb_l = np.tile(bout, (128, 4))

    in_maps = []
    for c in range(N_CORES):
        tr = trajectory[c * BC:(c + 1) * BC, :t_steps]    # [512, t, 64]
        ac = actions[c * BC:(c + 1) * BC, :t_steps]       # [512, t, 16]
        xt = np.zeros((128, NCH, t_steps, CB), ml_dtypes.bfloat16)
        xt[0:OBS] = tr.reshape(NCH, CB, t_steps, OBS).transpose(
            3, 0, 2, 1).astype(ml_dtypes.bfloat16)
        xt[OBS:OBS + ACTD] = ac.reshape(NCH, CB, t_steps, ACTD).transpose(
            3, 0, 2, 1).astype(ml_dtypes.bfloat16)
        xt[80] = 1.0
        in_maps.append({
            "xt": xt.reshape(128, NCH * t_steps * CB),
            "wh": wh_l, "wi": wi_l, "mlpw": mlpw_l,
            "mlpb": mlpb_l.astype(f32), "wout": wout_l,
            "boutb": boutb_l.astype(f32),
        })
    return in_maps


_RUNNER = {}


def _get_runner(t_steps=T):
    """Build the bass module once and wrap it in a cached, reusable
    shard-mapped PJRT executable (one NEFF compile per process)."""
    if t_steps in _RUNNER:
        return _RUNNER[t_steps]

    import jax
    from jax.sharding import Mesh, PartitionSpec
    from jax.experimental.shard_map import shard_map
    from concourse import bass2jax, mybir as _mb

    nc = _build(t_steps)
    bass2jax.install_neuronx_cc_hook()

    part_name = (nc.partition_id_tensor.name if nc.partition_id_tensor
                 else None)
    in_names, out_names, out_avals = [], [], []
    for alloc in nc.m.functions[0].allocations:
        if not isinstance(alloc, _mb.MemoryLocationSet):
            continue
        name = alloc.memorylocations[0].name
        if alloc.kind == "ExternalInput":
            if name != part_name:
                in_names.append(name)
        elif alloc.kind == "ExternalOutput":
            out_names.append(name)
            out_avals.append(jax.core.ShapedArray(
                tuple(alloc.tensor_shape), _mb.dt.np(alloc.dtype)))
    n_params = len(in_names)
    n_outs = len(out_avals)
    all_names = in_names + out_names
    if part_name is not None:
        all_names = all_names + [part_name]

    def _body(*args):
        operands = list(args)
        if part_name is not None:
            operands.append(bass2jax.partition_id_tensor())
        outs = bass2jax._bass_exec_p.bind(
            *operands,
            out_avals=tuple(out_avals),
            in_names=tuple(all_names),
            out_names=tuple(out_names),
            lowering_input_output_aliases=(),
            sim_require_finite=True,
            sim_require_nnan=True,
            nc=nc,
        )
        return tuple(outs)

    devices = jax.devices()[:N_CORES]
    mesh = Mesh(np.asarray(devices), ("core",))
    donate = tuple(range(n_params, n_params + n_outs))
    sharded = jax.jit(
        shard_map(_body, mesh=mesh,
                  in_specs=(PartitionSpec("core"),) * (n_params + n_outs),
                  out_specs=(PartitionSpec("core"),) * n_outs,
                  check_rep=False),
        donate_argnums=donate, keep_unused=True)

    sharded_nodon = jax.jit(
        shard_map(_body, mesh=mesh,
                  in_specs=(PartitionSpec("core"),) * (n_params + n_outs),
                  out_specs=(PartitionSpec("core"),) * n_outs,
                  check_rep=False),
        keep_unused=True)

    out_shapes = [(a.shape, a.dtype) for a in out_avals]

    def run(in_maps):
        concat_in = [
            np.concatenate([np.asarray(in_maps[c][nm]) for c in
                            range(N_CORES)], axis=0)
            for nm in in_names
        ]
        zeros = [np.zeros((N_CORES * s[0],) + tuple(s[1:]), dt)
                 for s, dt in out_shapes]
        outs = sharded(*concat_in, *zeros)
        return {nm: np.asarray(outs[i]) for i, nm in enumerate(out_names)}

    run.in_names = in_names
    run.mesh = mesh
    run.nodon = sharded_nodon
    run.out_shapes = out_shapes
    _RUNNER[t_steps] = run
    return run


def _stage_inputs(in_maps, t_steps=T):
    """device_put concatenated inputs + zero outs once, for repeat timing."""
    import jax
    from jax.sharding import NamedSharding, PartitionSpec
    run = _get_runner(t_steps)
    sh = NamedSharding(run.mesh, PartitionSpec("core"))
    concat_in = [
        np.concatenate([np.asarray(in_maps[c][nm]) for c in range(N_CORES)],
                       axis=0)
        for nm in run.in_names
    ]
    zeros = [np.zeros((N_CORES * s[0],) + tuple(s[1:]), dt)
             for s, dt in run.out_shapes]
    return [jax.device_put(a, sh) for a in concat_in + zeros], run


def _run_staged(staged):
    arrs, run = staged
    return run.nodon(*arrs)


def kernel(trajectory, actions, Wi, Wh, bh, mlp_W, mlp_b, Wout, bout):
    run = _get_runner(T)
    in_maps = _prep_inputs(trajectory, actions, Wi, Wh, bh, mlp_W, mlp_b,
                           Wout, bout, T)
    pred = run(in_maps)["pred"]          # [8*512, 64] already batch-ordered
    return pred.astype(np.float32)
